# revision 9
# baseline (speedup 1.0000x reference)
"""MixtureOfDepths router kernel for 8 Trainium2 NeuronCores.

Problem (hardcoded shapes): hidden_states (4, 8192, 4096) f32, router weight
w (4096,) f32, bias b () f32.
  logits = hidden_states @ w + b        (4, 8192)
  weights = sigmoid(logits)
  k = 4096; threshold = k-th largest weight per batch row
  mask = weights >= threshold

Sharding: core c handles batch c//2, sequence half c%2 -> a (4096, 4096)
slice (64 MiB).  Per core: 32 tiles of [128 tokens x 4096 hidden], one DVE
scalar_tensor_tensor (mult + add-reduce) per tile -> logits [128, 32];
per-tile ACT sigmoid -> weights.

Threshold (exact k-th largest, via bit-space counting; positive floats are
order-isomorphic to their int32 bit patterns):
  * During the stream, each tile's 128 weights are compared against a fixed
    1024-point grid (step 2^20 in bit space) with one DVE tensor_scalar
    (is_le, bf16 out); a PE matmul with a ones-stationary accumulates the
    compare matrix column-sums into PSUM -> exact per-candidate counts for
    all local weights, at ~zero marginal cost (PE is otherwise idle).
  * Pairwise AllGather shares [weights | counts] (one 5120-f32 buffer).
  * Merged counts localize the k-th value to a 2^20-wide bit bracket.
  * The gathered 8192 weights are replicated only 32-way (partition group g
    holds data quarter g), so each bisect round is a [128, 2048] DVE scan
    (radix 32: candidate p%32 on quarter p//32) plus two gpsimd partition-
    slice adds.  Four rounds (steps 2^15, 2^10, 2^5, 1) finish the exact
    threshold.  Ties handled exactly like the reference (mask = w >= kth).

Engine notes (HW-verified in a previous session): DVE int32 adds at
magnitude 2^30 round to 64s (fp32-internal ALU), so all bit-space integer
adds run on gpsimd (Q7, exact).  tensor_tensor_reduce crashes at runtime;
the InstTensorScalarPtr family is HW-verified.
"""

import sys

if "/opt/trn_rl_repo" not in sys.path:
    sys.path.insert(0, "/opt/trn_rl_repo")

from contextlib import ExitStack

import numpy as np

import concourse.bass as bass  # noqa: F401  (bass types via bacc)
import concourse.tile as tile
from concourse import bacc, mybir
from concourse import bass_isa
from concourse import bass2jax
from concourse import mybir as _mb

N_CORES = 8
BATCH = 4
SEQ = 8192
HIDDEN = 4096
K = SEQ // 2  # 4096

NGRID = 1024          # fixed candidate grid, step 2^20 in bit space
GRID_STEP = 1 << 20
ROUND_STEPS = [1 << 15, 1 << 10, 1 << 5, 1]  # radix-32 rounds after the grid


def build(n_cores=N_CORES, tok=SEQ // 2, hidden=HIDDEN, k=K, pair_groups=None,
          fake_gather=False, hbufs=4):
    """Build the SPMD bass module."""
    f32, i32, u8 = mybir.dt.float32, mybir.dt.int32, mybir.dt.uint8
    bf16 = mybir.dt.bfloat16
    ntile = tok // 128
    assert tok % 128 == 0
    if pair_groups is None:
        pair_groups = [[2 * i, 2 * i + 1] for i in range(n_cores // 2)]

    nc = bacc.Bacc("TRN2", target_bir_lowering=False, debug=False,
                   num_devices=n_cores)

    nrounds = len(ROUND_STEPS)
    hs = nc.dram_tensor("hs", [tok, hidden], f32, kind="ExternalInput").ap()
    w1 = nc.dram_tensor("w1", [1, hidden], f32, kind="ExternalInput").ap()
    bias2 = nc.dram_tensor("bias2", [128, 1], f32, kind="ExternalInput").ap()
    # io3[p, r] = (p % 32) * ROUND_STEPS[r]  (host-precomputed)
    io3 = nc.dram_tensor("io3", [128, nrounds], i32, kind="ExternalInput").ap()
    # candgrid[p, c] = float32_bitcast(c * GRID_STEP), identical rows
    cgrid = nc.dram_tensor("cgrid", [128, NGRID], f32, kind="ExternalInput").ap()
    ones2 = nc.dram_tensor("ones2", [128, 1], f32, kind="ExternalInput").ap()
    wout = nc.dram_tensor("wout", [128, ntile], f32, kind="ExternalOutput").ap()
    mout = nc.dram_tensor("mout", [128, ntile], u8, kind="ExternalOutput").ap()

    # token t = p * ntile + n  ->  partition p, tile-slot n
    hs3 = hs.rearrange("(p n) d -> p n d", p=128)

    GW = ntile * 128            # 4096 local weights
    GTOT = GW + NGRID           # 5120 = weights + grid counts

    with tile.TileContext(nc) as tc, ExitStack() as ctx:
        consts = ctx.enter_context(tc.tile_pool(name="consts", bufs=1))
        hpool = ctx.enter_context(tc.tile_pool(name="hid", bufs=hbufs))
        spool = ctx.enter_context(tc.tile_pool(name="big", bufs=1))
        cmp_pool = ctx.enter_context(tc.tile_pool(name="cmp", bufs=3))
        small = ctx.enter_context(tc.tile_pool(name="small", bufs=1))
        psum = ctx.enter_context(tc.tile_pool(name="psum", bufs=1, space="PSUM"))
        dram = ctx.enter_context(tc.tile_pool(name="dram", bufs=1, space="DRAM"))

        # ---- constants ----
        w1b = consts.tile([1, hidden], f32)
        nc.scalar.dma_start(out=w1b[:], in_=w1[:])
        wb = consts.tile([128, hidden], f32)
        nc.gpsimd.partition_broadcast(wb[:], w1b[:], channels=128)
        bb = consts.tile([128, 1], f32)
        nc.scalar.dma_start(out=bb[:], in_=bias2[:])
        io = consts.tile([128, nrounds], i32)
        nc.scalar.dma_start(out=io[:], in_=io3[:])
        cgf = consts.tile([128, NGRID], f32)
        nc.scalar.dma_start(out=cgf[:], in_=cgrid[:])
        cgb = consts.tile([128, NGRID], bf16)
        # exact: grid bit patterns have zero low-16 bits
        nc.vector.tensor_scalar(out=cgb[:], in0=cgf[:], scalar1=0.0,
                                scalar2=None, op0=mybir.AluOpType.add)
        onesf = consts.tile([128, 1], f32)
        nc.scalar.dma_start(out=onesf[:], in_=ones2[:])
        onesb = consts.tile([128, 1], bf16)
        nc.vector.tensor_scalar(out=onesb[:], in0=onesf[:], scalar1=0.0,
                                scalar2=None, op0=mybir.AluOpType.add)

        NA = ntile - 1  # 31 tiles gathered early; the last rides with the counts
        logits = small.tile([128, ntile], f32, tag="logits")
        wsigA = small.tile([128, NA], f32, tag="wsigA")
        wsigB = small.tile([128, 1], f32, tag="wsigB")
        cntA = psum.tile([1, NGRID // 2], f32, tag="cntA")
        cntB = psum.tile([1, NGRID // 2], f32, tag="cntB")

        # ---- stream: dot, sigmoid, grid-count per tile ----
        for i in range(ntile):
            ht = hpool.tile([128, hidden], f32, tag="ht")
            dma_eng = nc.sync if i % 2 == 0 else nc.scalar
            dma_eng.dma_start(out=ht[:], in_=hs3[:, i, :])
            sc = spool.tile([128, hidden], f32, tag="sc")
            nc.vector.scalar_tensor_tensor(
                out=sc[:], in0=ht[:], scalar=1.0, in1=wb[:],
                op0=mybir.AluOpType.mult, op1=mybir.AluOpType.mult,
                accum_out=logits[:, i:i + 1])
            wcol = wsigA[:, i:i + 1] if i < NA else wsigB[:]
            nc.scalar.activation(out=wcol, in_=logits[:, i:i + 1],
                                 func=mybir.ActivationFunctionType.Sigmoid,
                                 bias=bb[:])
            # S[p, c] = (grid_c <= w_p)  -> column sums accumulate in PSUM.
            # Tiles 0..30 compare on gpsimd (DVE is nearly saturated by the
            # dots); the last tile's compare runs on DVE - it is on the
            # critical path and DVE is free by then.
            st = cmp_pool.tile([128, NGRID], bf16, tag="st")
            cmp_eng = nc.gpsimd if i < NA else nc.vector
            cmp_eng.tensor_scalar(
                out=st[:], in0=cgb[:], scalar1=wcol, scalar2=None,
                op0=mybir.AluOpType.is_le)
            nc.tensor.matmul(cntA[:], onesb[:], st[:, 0:NGRID // 2],
                             start=(i == 0), stop=(i == ntile - 1))
            nc.tensor.matmul(cntB[:], onesb[:], st[:, NGRID // 2:],
                             start=(i == 0), stop=(i == ntile - 1))
            if i == NA - 1:
                # early gather of tiles 0..30 (both weights halves), issued
                # while the last tile is still streaming/compounding
                gin1 = dram.tile([1, NA * 128], f32)
                gw1 = gin1[:].rearrange("a (p n) -> (a p) n", p=128)
                nc.sync.dma_start(out=gw1, in_=wsigA[:])
                gout1 = dram.tile([1, 2 * NA * 128], f32)
                if fake_gather:
                    g2 = gout1[:].rearrange("a (h t) -> a h t", h=2)
                    nc.sync.dma_start(out=g2[:, 0, :], in_=gin1.opt())
                    nc.sync.dma_start(out=g2[:, 1, :], in_=gin1.opt())
                else:
                    nc.gpsimd.collective_compute(
                        "AllGather", mybir.AluOpType.bypass,
                        replica_groups=pair_groups,
                        ins=[gin1.opt()], outs=[gout1.opt()])

        nc.sync.dma_start(out=wout[:, 0:NA], in_=wsigA[:])
        nc.scalar.dma_start(out=wout[:, NA:], in_=wsigB[:])

        # ---- local grid counts PSUM -> SBUF -> DRAM; AllReduce-add merges ----
        cloc = small.tile([1, NGRID], f32, tag="cloc")
        nc.vector.tensor_scalar(out=cloc[:, 0:NGRID // 2], in0=cntA[:],
                                scalar1=0.0, scalar2=None,
                                op0=mybir.AluOpType.add)
        nc.scalar.copy(out=cloc[:, NGRID // 2:], in_=cntB[:])

        gin2 = dram.tile([1, 128], f32)
        gw2 = gin2[:].rearrange("a (p n) -> (a p) n", p=128)
        nc.scalar.dma_start(out=gw2, in_=wsigB[:])
        gout2 = dram.tile([1, 256], f32)
        ginc = dram.tile([1, NGRID], f32)
        nc.scalar.dma_start(out=ginc[:], in_=cloc[:])
        goutc = dram.tile([1, NGRID], f32)
        if fake_gather:
            g2b = gout2[:].rearrange("a (h t) -> a h t", h=2)
            nc.scalar.dma_start(out=g2b[:, 0, :], in_=gin2.opt())
            nc.scalar.dma_start(out=g2b[:, 1, :], in_=gin2.opt())
            nc.scalar.dma_start(out=goutc[:], in_=ginc.opt())
        else:
            nc.gpsimd.collective_compute(
                "AllGather", mybir.AluOpType.bypass,
                replica_groups=pair_groups,
                ins=[gin2.opt()], outs=[gout2.opt()])
            nc.gpsimd.collective_compute(
                "AllReduce", mybir.AluOpType.add,
                replica_groups=pair_groups,
                ins=[ginc.opt()], outs=[goutc.opt()])

        # ---- wall3: 32-way replication via stride-0 DMA broadcast reads ----
        # partition group g holds data quarter g; order within a row is free
        QW = GW // 2    # 2048
        NAW = NA * 128  # 3968
        HQ = NAW - QW   # 1920: early-gather part of quarters 1 and 3
        wall3 = spool.tile([128, QW], f32, tag="wall3")
        nc.sync.dma_start(out=wall3[0:32, :],
                          in_=gout1[:, 0:QW].broadcast_to((32, QW)))
        nc.scalar.dma_start(out=wall3[32:64, 0:HQ],
                            in_=gout1[:, QW:NAW].broadcast_to((32, HQ)))
        nc.sync.dma_start(out=wall3[64:96, :],
                          in_=gout1[:, NAW:NAW + QW].broadcast_to((32, QW)))
        nc.scalar.dma_start(out=wall3[96:128, 0:HQ],
                            in_=gout1[:, NAW + QW:2 * NAW].broadcast_to((32, HQ)))
        nc.sync.dma_start(out=wall3[32:64, HQ:QW],
                          in_=gout2[:, 0:128].broadcast_to((32, 128)))
        nc.scalar.dma_start(out=wall3[96:128, HQ:QW],
                            in_=gout2[:, 128:256].broadcast_to((32, 128)))

        # ---- merged grid counts; base0 = (nb - 1) * GRID_STEP ----
        cmrg = small.tile([1, NGRID], f32, tag="cmrg")
        nc.sync.dma_start(out=cmrg[:], in_=goutc[:])
        cflag = small.tile([1, NGRID], f32, tag="cflag")
        nb = small.tile([1, 1], f32, tag="nb")
        nc.vector.tensor_scalar(out=cflag[:], in0=cmrg[:], scalar1=float(k),
                                scalar2=None, op0=mybir.AluOpType.is_ge,
                                op1=mybir.AluOpType.add, accum_out=nb[:])
        base0 = small.tile([1, 1], i32, tag="base0")
        nc.gpsimd.tensor_scalar(out=base0[:], in0=nb[:], scalar1=1.0,
                                scalar2=float(GRID_STEP),
                                op0=mybir.AluOpType.subtract,
                                op1=mybir.AluOpType.mult)
        base = small.tile([128, 1], i32, tag="baseA")
        nc.gpsimd.partition_broadcast(base[:], base0[:], channels=128)
        base_alt = small.tile([128, 1], i32, tag="baseB")

        # ---- 4 radix-32 rounds ----
        cnt4 = small.tile([128, 1], f32, tag="cnt4")
        tmp64 = small.tile([128, 1], f32, tag="tmp64")
        s64 = small.tile([128, 1], f32, tag="s64")
        tmp32 = small.tile([128, 1], f32, tag="tmp32")
        cntp = small.tile([128, 1], f32, tag="cntp")
        nc.vector.memset(cntp[:], 0)
        flag = small.tile([128, 1], f32, tag="flag")
        sumf = small.tile([128, 1], f32, tag="sumf")
        delta = small.tile([128, 1], i32, tag="delta")
        csc = spool.tile([128, QW], f32, tag="csc")

        for r, s in enumerate(ROUND_STEPS):
            cand = small.tile([128, 1], i32, tag="cand")
            nc.gpsimd.tensor_add(cand[:], io[:, r:r + 1], base[:])
            nc.vector.tensor_scalar(
                out=csc[:], in0=wall3[:], scalar1=cand[:].bitcast(f32),
                scalar2=None, op0=mybir.AluOpType.is_ge,
                op1=mybir.AluOpType.add, accum_out=cnt4[:])
            # quarter merge: TensorTensor needs equal base partitions, so
            # cross-partition moves go through single-input copies
            nc.gpsimd.tensor_copy(tmp64[0:64, :], cnt4[64:128, :])
            nc.gpsimd.tensor_add(s64[0:64, :], cnt4[0:64, :], tmp64[0:64, :])
            nc.gpsimd.tensor_copy(tmp32[0:32, :], s64[32:64, :])
            nc.gpsimd.tensor_add(cntp[0:32, :], s64[0:32, :], tmp32[0:32, :])
            nc.vector.tensor_scalar(
                out=flag[:], in0=cntp[:], scalar1=float(k), scalar2=None,
                op0=mybir.AluOpType.is_ge)
            nc.gpsimd.partition_all_reduce(
                sumf[:], flag[:], channels=128,
                reduce_op=bass_isa.ReduceOp.add)
            nc.vector.tensor_scalar(
                out=delta[:], in0=sumf[:], scalar1=1.0, scalar2=float(s),
                op0=mybir.AluOpType.subtract, op1=mybir.AluOpType.mult)
            nc.gpsimd.tensor_add(base_alt[:], delta[:], base[:])
            base, base_alt = base_alt, base

        # ---- mask: own weights >= threshold (exact k-th largest value) ----
        mask = small.tile([128, ntile], u8, tag="mask")
        nc.vector.tensor_scalar(
            out=mask[:, 0:NA], in0=wsigA[:], scalar1=base[:].bitcast(f32),
            scalar2=None, op0=mybir.AluOpType.is_ge)
        nc.vector.tensor_scalar(
            out=mask[:, NA:], in0=wsigB[:], scalar1=base[:].bitcast(f32),
            scalar2=None, op0=mybir.AluOpType.is_ge)
        nc.sync.dma_start(out=mout[:], in_=mask[:])

    nc.compile()
    return nc


class Runner:
    """Executes a built Bass module on the 8 axon NeuronCores via PJRT,
    building the sharded jit executable once and reusing it."""

    def __init__(self, nc, n_cores=N_CORES):
        import jax
        from jax.sharding import Mesh, PartitionSpec
        from jax.experimental.shard_map import shard_map

        bass2jax.install_neuronx_cc_hook()
        self.n_cores = n_cores
        partition_name = (nc.partition_id_tensor.name
                          if nc.partition_id_tensor else None)
        in_names, out_names, out_avals, zero_outs = [], [], [], []
        for alloc in nc.m.functions[0].allocations:
            if not isinstance(alloc, _mb.MemoryLocationSet):
                continue
            name = alloc.memorylocations[0].name
            if alloc.kind == "ExternalInput":
                if name != partition_name:
                    in_names.append(name)
            elif alloc.kind == "ExternalOutput":
                shape = tuple(alloc.tensor_shape)
                dtype = _mb.dt.np(alloc.dtype)
                out_names.append(name)
                out_avals.append(jax.core.ShapedArray(shape, dtype))
                zero_outs.append(np.zeros(shape, dtype))
        self.in_names, self.out_names = list(in_names), out_names
        self.out_avals, self.zero_outs = out_avals, zero_outs
        n_params, n_outs = len(in_names), len(out_avals)
        self.n_params = n_params
        all_names = in_names + out_names
        if partition_name is not None:
            all_names = all_names + [partition_name]

        def _body(*args):
            operands = list(args)
            if partition_name is not None:
                operands.append(bass2jax.partition_id_tensor())
            return tuple(bass2jax._bass_exec_p.bind(
                *operands,
                out_avals=tuple(out_avals),
                in_names=tuple(all_names),
                out_names=tuple(out_names),
                lowering_input_output_aliases=(),
                sim_require_finite=True,
                sim_require_nnan=True,
                nc=nc,
            ))

        devices = jax.devices()[:n_cores]
        self.mesh = Mesh(np.asarray(devices), ("core",))
        self.pspec = PartitionSpec("core")
        in_specs = (self.pspec,) * (n_params + n_outs)
        out_specs = (self.pspec,) * n_outs
        self.sharded = jax.jit(
            shard_map(_body, mesh=self.mesh, in_specs=in_specs,
                      out_specs=out_specs, check_rep=False),
            donate_argnums=tuple(range(n_params, n_params + n_outs)),
            keep_unused=True)

    def concat_inputs(self, in_maps):
        return [np.concatenate([np.asarray(in_maps[c][nm])
                                for c in range(self.n_cores)], axis=0)
                for nm in self.in_names]

    def fresh_zeros(self):
        return [np.zeros((self.n_cores * z.shape[0], *z.shape[1:]), z.dtype)
                for z in self.zero_outs]

    def call(self, concat_in):
        return self.sharded(*concat_in, *self.fresh_zeros())

    def run(self, in_maps):
        out_arrs = self.call(self.concat_inputs(in_maps))
        return [
            {nm: np.asarray(out_arrs[i]).reshape(
                self.n_cores, *self.out_avals[i].shape)[c]
             for i, nm in enumerate(self.out_names)}
            for c in range(self.n_cores)
        ]


_NC_CACHE = {}


def _get_nc():
    if "full" not in _NC_CACHE:
        _NC_CACHE["full"] = build()
    return _NC_CACHE["full"]


def _get_runner():
    if "runner" not in _NC_CACHE:
        _NC_CACHE["runner"] = Runner(_get_nc())
    return _NC_CACHE["runner"]


def make_in_maps(hidden_states, w, b, n_cores=N_CORES, tok=SEQ // 2):
    hs = np.asarray(hidden_states, dtype=np.float32)
    wv = np.asarray(w, dtype=np.float32).reshape(1, -1)
    bias2 = np.full((128, 1), np.float32(b), dtype=np.float32)
    io3 = ((np.arange(128, dtype=np.int64) % 32)[:, None]
           * np.asarray(ROUND_STEPS, dtype=np.int64)[None, :]).astype(np.int32)
    grid_bits = (np.arange(NGRID, dtype=np.int64) * GRID_STEP).astype(np.int32)
    grid_f = grid_bits.view(np.float32)
    cgrid = np.ascontiguousarray(
        np.broadcast_to(grid_f[None, :], (128, NGRID)))
    ones2 = np.ones((128, 1), dtype=np.float32)
    in_maps = []
    for c in range(n_cores):
        bb, h = c // 2, c % 2
        shard = np.ascontiguousarray(hs[bb, h * tok:(h + 1) * tok, :])
        in_maps.append({"hs": shard, "w1": wv, "bias2": bias2, "io3": io3,
                        "cgrid": cgrid, "ones2": ones2})
    return in_maps


def assemble(results, n_cores=N_CORES, tok=SEQ // 2):
    weights = np.empty((BATCH, SEQ), dtype=np.float32)
    mask = np.empty((BATCH, SEQ), dtype=bool)
    for c in range(n_cores):
        bb, h = c // 2, c % 2
        weights[bb, h * tok:(h + 1) * tok] = results[c]["wout"].reshape(-1)
        mask[bb, h * tok:(h + 1) * tok] = results[c]["mout"].reshape(-1) != 0
    return weights, mask


def kernel(hidden_states, w, b):
    runner = _get_runner()
    in_maps = make_in_maps(hidden_states, w, b)
    return assemble(runner.run(in_maps))
